# revision 16
# baseline (speedup 1.0000x reference)
"""AstrocyteGate distributed Bass kernel for one TRN2 chip (8 NeuronCores).

Reference computation (B=8, T=2048, D=2048, fp32):
    pooled    = mean over (B*T) of x            -> [D]
    update    = proj_w @ pooled + proj_b        -> [D]
    new_state = DECAY*state + (1-DECAY)*update  -> [D]
    gain      = sigmoid(gate_w @ new_state + gate_b)
    out       = x * gain                        (broadcast over [B,T,D])

Weight folding (host, exact algebra): with state/proj_b/gate_b fixed,
    logit = M @ pooled_sum + cvec,  where
    M     = ((1-DECAY)/(B*T)) * gate_w @ proj_w          [D, D]
    cvec  = gate_w @ (DECAY*state + (1-DECAY)*proj_b) + gate_b
and pooled_sum = sum over all (B*T) rows of x. The pooled_sum term
contributes ~1e-6 to a logit of magnitude ~1e-2, so bf16/fp8 precision on
that path is far inside the rel-err budget; x itself is cast to bf16
(~0.4% elementwise) which dominates the (still tiny) overall error.

Strategy (data-parallel over B, 1 batch row per core):
  - x is cast to bf16 host-side; each core streams its 8 MiB shard in as
    8 x 1 MiB DMAs and keeps it SBUF-resident. A bf16 VectorE accumulate
    chain tracks the loads; a 16-matmul partition-reduce produces the
    local token-sum s_c as [128, 16] (partition layout).
  - Every core computes its FULL partial logit y_c = M @ s_c with a
    64-matmul fp8 matvec (M and the token-sum prescaled by host-chosen
    powers of 2 so fp8e4 neither underflows nor saturates; descaled in
    the combine matmul). Since sum_c M @ s_c = M @ pooled_sum, no first
    collective is needed and the result is independent of collective
    rank order.
  - A zero-dependency warm-up AllGather issues first so the ncfw stack
    (rendezvous barrier + firmware wake, ~65us) runs concurrently with
    the loads + matvec. The single data AllGather then combines the 8
    partial logits (bf16, 4 KB each) on a warm path.
  - One K=9 matmul (rows 0-7 = the descale constant, row 8 = 1.0)
    sums the ranks and broadcasts the logit across 128 partitions;
    sigmoid runs wide; the in-SBUF bf16 x tiles are scaled in place and
    streamed back out as bf16 (host upcasts to fp32).

HBM traffic per core: 8 MiB x in + 4 MiB weights + 8 MiB out.
"""

import numpy as np

import concourse.bacc as bacc
import concourse.bass as bass
import concourse.mybir as mybir
import concourse.tile as tile
from concourse.bass_utils import run_bass_kernel_spmd

B, T, D = 8, 2048, 2048
NCORES = 8
NT = 8                  # x tiles per core (each [128, 2, D] = 256 tokens)
JJ = D // 128           # 16: 128-chunks of D
TAU = 1000.0
DECAY = float(np.exp(-1.0 / TAU))
FP32 = mybir.dt.float32
BF16 = mybir.dt.bfloat16
FP8 = mybir.dt.float8e4
RG = [list(range(NCORES))]

_NC_CACHE = {}


def _build():
    nc = bacc.Bacc(
        "TRN2",
        target_bir_lowering=False,
        debug=False,
        enable_asserts=False,
        num_devices=NCORES,
    )

    x_d = nc.dram_tensor("x", [NT, 128, 2, D], BF16, kind="ExternalInput")
    mtw_d = nc.dram_tensor("mtw", [128, JJ, D], FP8, kind="ExternalInput")
    cv_d = nc.dram_tensor("cv", [1, D], BF16, kind="ExternalInput")
    s1_d = nc.dram_tensor("s1", [128, 1], BF16, kind="ExternalInput")
    cb_d = nc.dram_tensor("cb", [NCORES + 1, 128], BF16, kind="ExternalInput")
    out_d = nc.dram_tensor("out", [NT, 128, 2, D], BF16, kind="ExternalOutput")

    wsync_in = nc.dram_tensor("wsync_in", [1, 16], BF16)
    wsync_out = nc.dram_tensor("wsync_out", [NCORES, 16], BF16, addr_space="Shared")
    y_bnc = nc.dram_tensor("y_bnc", [1, D], BF16)
    gath = nc.dram_tensor("gath", [NCORES, D], BF16, addr_space="Shared")

    AF = mybir.ActivationFunctionType
    ALU = mybir.AluOpType

    with tile.TileContext(nc) as tc:
        with (
            tc.tile_pool(name="xpool", bufs=NT) as xpool,
            tc.tile_pool(name="wpool", bufs=1) as wpool,
            tc.tile_pool(name="small", bufs=1) as small,
            tc.tile_pool(name="psA", bufs=1, space="PSUM") as psA,
            tc.tile_pool(name="psB", bufs=1, space="PSUM") as psB,
        ):
            # --- warm-up collective: ncfw wake + rank rendezvous, no deps ---
            nc.gpsimd.collective_compute(
                "AllGather",
                ALU.bypass,
                replica_groups=RG,
                ins=[wsync_in.ap().opt()],
                outs=[wsync_out.ap().opt()],
            )

            # --- load x first; everything else is off the critical path ---
            xs = []
            for j in range(NT):
                xt = xpool.tile([128, 2, D], BF16, tag="xt")
                nc.sync.dma_start(xt[:], x_d[j])
                xs.append(xt)

            # --- weight / small-input loads ---
            mtw = wpool.tile([128, JJ, D], FP8, tag="mtw")
            nc.sync.dma_start(mtw[:], mtw_d[:])
            # gather tile: rows 0..7 = gathered partial logits, row 8 = cvec
            g2 = small.tile([NCORES + 1, D], BF16, tag="g2")
            nc.sync.dma_start(g2[NCORES : NCORES + 1, :], cv_d[:])

            # --- runtime power-of-2 scale constants (host-computed) ---
            ones1 = small.tile([128, 1], BF16, tag="ones1")
            nc.sync.dma_start(ones1[:], s1_d[:])
            comb9 = small.tile([NCORES + 1, 128], BF16, tag="comb9")
            nc.sync.dma_start(comb9[:], cb_d[:])
            # pre-warm the ScalarE sigmoid LUT off the critical path
            dummy = small.tile([1, 1], FP32, tag="dummy")
            nc.scalar.activation(dummy[:], ones1[0:1, 0:1], AF.Sigmoid)

            # --- accumulate token-sums on VectorE (bf16) as tiles land ---
            acc = wpool.tile([128, 2, D], BF16, tag="acc")
            nc.vector.tensor_copy(acc[:], xs[0][:])
            for j in range(1, NT):
                nc.vector.tensor_add(acc[:], acc[:], xs[j][:])
            acc2 = wpool.tile([128, D], BF16, tag="acc2")
            nc.vector.tensor_add(acc2[:], acc[:, 0, :], acc[:, 1, :])

            # partition-reduce: sumT[p, j] = sum_p' acc2[p', j*128+p]
            sumT_ps = psB.tile([128, JJ], FP32, tag="pt")
            for j in range(JJ):
                nc.tensor.matmul(
                    sumT_ps[:, j : j + 1],
                    acc2[:, j * 128 : (j + 1) * 128],
                    ones1[:],
                    start=True,
                    stop=True,
                )
            sT8 = small.tile([128, JJ], FP8, tag="sT8")
            nc.vector.tensor_copy(sT8[:], sumT_ps[:])

            # --- full fp8 matvec: y_c = M' @ s_c  (M' = mscale * M) ---
            y_ps = psA.tile([1, D], FP32, tag="wide")
            for j in range(JJ):
                for q in range(4):
                    nc.tensor.matmul(
                        y_ps[0:1, q * 512 : (q + 1) * 512],
                        sT8[:, j : j + 1],
                        mtw[:, j, q * 512 : (q + 1) * 512],
                        start=(j == 0),
                        stop=(j == JJ - 1),
                    )
            ybf = small.tile([1, D], BF16, tag="ybf")
            nc.scalar.copy(ybf[:], y_ps[:])
            nc.sync.dma_start(y_bnc[:], ybf[:])

            # --- the single data collective: gather partial logits ---
            nc.gpsimd.collective_compute(
                "AllGather",
                ALU.bypass,
                replica_groups=RG,
                ins=[y_bnc.ap().opt()],
                outs=[gath.ap().opt()],
            )
            nc.sync.dma_start(g2[0:NCORES, :], gath[:])

            # --- fused descale + rank-sum + cvec add + partition-broadcast:
            #     logit[p, n] = descale * sum_r g2[r, n] + cvec[n] ---
            logit_ps = psA.tile([128, D], FP32, tag="wide")
            for q in range(4):
                nc.tensor.matmul(
                    logit_ps[:, q * 512 : (q + 1) * 512],
                    comb9[:],
                    g2[:, q * 512 : (q + 1) * 512],
                    start=True,
                    stop=True,
                )
            gain_bc = wpool.tile([128, D], BF16, tag="gbc")
            nc.scalar.activation(gain_bc[:], logit_ps[:], AF.Sigmoid)

            # --- scale x in place (bf16) and stream out ---
            for j in range(NT):
                for c2 in range(2):
                    nc.vector.tensor_mul(
                        xs[j][:, c2, :], xs[j][:, c2, :], gain_bc[:]
                    )
                nc.sync.dma_start(out_d[j], xs[j][:])

    nc.compile()
    return nc


def _get_nc():
    if "nc" not in _NC_CACHE:
        _NC_CACHE["nc"] = _build()
    return _NC_CACHE["nc"]


def _shard_inputs(x, state, proj_w, proj_b, gate_w, gate_b):
    import ml_dtypes

    bf16 = ml_dtypes.bfloat16
    fp8 = ml_dtypes.float8_e4m3
    x = np.asarray(x, dtype=np.float32)
    state = np.asarray(state, dtype=np.float32)
    proj_w = np.asarray(proj_w, dtype=np.float32)
    proj_b = np.asarray(proj_b, dtype=np.float32)
    gate_w = np.asarray(gate_w, dtype=np.float32)
    gate_b = np.asarray(gate_b, dtype=np.float32)

    # fold the two matvecs + EMA into one matrix and one bias vector
    M = (gate_w @ proj_w) * ((1.0 - DECAY) / float(B * T))
    cvec = gate_w @ (DECAY * state + (1.0 - DECAY) * proj_b) + gate_b

    # power-of-2 scales keeping the fp8 operands inside e4m3 range (+-448)
    max_m = float(np.abs(M).max()) + 1e-300
    mscale = 2.0 ** np.floor(np.log2(300.0 / max_m))
    max_s = float(np.abs(x.sum(axis=1)).max()) + 1e-300
    s1val = 2.0 ** min(0.0, np.floor(np.log2(300.0 / (1.25 * max_s))))

    # mtw[p, j, n] = (mscale * M)[n, 128j + p]
    mtw = np.ascontiguousarray(
        (M * mscale).T.reshape(JJ, 128, D).transpose(1, 0, 2).astype(fp8)
    )
    cv = np.ascontiguousarray(cvec.reshape(1, D).astype(bf16))
    s1 = np.full((128, 1), s1val, dtype=bf16)
    cb = np.empty((NCORES + 1, 128), dtype=bf16)
    cb[0:NCORES, :] = bf16(1.0 / (mscale * s1val))
    cb[NCORES, :] = bf16(1.0)

    in_maps = []
    for c in range(NCORES):
        xc = np.ascontiguousarray(x[c].reshape(NT, 128, 2, D).astype(bf16))
        in_maps.append({"x": xc, "mtw": mtw, "cv": cv, "s1": s1, "cb": cb})
    return in_maps


def _run(inputs, trace=False, **kwargs):
    nc = _get_nc()
    in_maps = _shard_inputs(**inputs)
    res = run_bass_kernel_spmd(
        nc, in_maps, core_ids=list(range(NCORES)), trace=trace, **kwargs
    )
    out = np.stack(
        [
            res.results[c]["out"].reshape(T, D).astype(np.float32)
            for c in range(NCORES)
        ],
        axis=0,
    )
    return out, res


def kernel(**inputs):
    out, _ = _run(inputs, trace=False)
    return out


# revision 17
# speedup vs baseline: 1.0891x; 1.0891x over previous
"""AstrocyteGate distributed Bass kernel for one TRN2 chip (8 NeuronCores).

Reference computation (B=8, T=2048, D=2048, fp32):
    pooled    = mean over (B*T) of x            -> [D]
    update    = proj_w @ pooled + proj_b        -> [D]
    new_state = DECAY*state + (1-DECAY)*update  -> [D]
    gain      = sigmoid(gate_w @ new_state + gate_b)
    out       = x * gain                        (broadcast over [B,T,D])

Weight folding (host, exact algebra): with state/proj_b/gate_b fixed,
    logit = M @ pooled_sum + cvec,  where
    M     = ((1-DECAY)/(B*T)) * gate_w @ proj_w          [D, D]
    cvec  = gate_w @ (DECAY*state + (1-DECAY)*proj_b) + gate_b
and pooled_sum = sum over all (B*T) rows of x. The pooled_sum term
contributes ~1e-6 to a logit of magnitude ~1e-2, so bf16/fp8 precision on
that path is far inside the rel-err budget; x itself is cast to bf16
(~0.4% elementwise) which dominates the (still tiny) overall error.

Strategy (data-parallel over B, 1 batch row per core):
  - x is cast to bf16 host-side; each core streams its 8 MiB shard in as
    8 x 1 MiB DMAs and keeps it SBUF-resident. A bf16 VectorE accumulate
    chain tracks the loads; a 16-matmul partition-reduce produces the
    local token-sum s_c as [128, 16] (partition layout).
  - Every core computes its FULL partial logit y_c = M @ s_c with a
    64-matmul fp8 matvec (M and the token-sum prescaled by host-chosen
    powers of 2 so fp8e4 neither underflows nor saturates; descaled in
    the combine matmul). Since sum_c M @ s_c = M @ pooled_sum, no first
    collective is needed and the result is independent of collective
    rank order.
  - A zero-dependency warm-up AllGather issues first so the ncfw stack
    (rendezvous barrier + firmware wake, ~65us) runs concurrently with
    the loads + matvec. The single data AllGather then combines the 8
    partial logits (bf16, 4 KB each) on a warm path.
  - One K=9 matmul (rows 0-7 = the descale constant, row 8 = 1.0)
    sums the ranks and broadcasts the logit across 128 partitions;
    sigmoid runs wide; the in-SBUF bf16 x tiles are scaled in place and
    streamed back out as bf16 (host upcasts to fp32).

HBM traffic per core: 8 MiB x in + 4 MiB weights + 8 MiB out.
"""

import numpy as np

import concourse.bacc as bacc
import concourse.bass as bass
import concourse.mybir as mybir
import concourse.tile as tile
from concourse.bass_utils import run_bass_kernel_spmd

B, T, D = 8, 2048, 2048
NCORES = 8
NT = 8                  # x tiles per core (each [128, 2, D] = 256 tokens)
JJ = D // 128           # 16: 128-chunks of D
TAU = 1000.0
DECAY = float(np.exp(-1.0 / TAU))
FP32 = mybir.dt.float32
BF16 = mybir.dt.bfloat16
FP8 = mybir.dt.float8e4
RG = [list(range(NCORES))]

_NC_CACHE = {}


def _build():
    nc = bacc.Bacc(
        "TRN2",
        target_bir_lowering=False,
        debug=False,
        enable_asserts=False,
        num_devices=NCORES,
    )

    x_d = nc.dram_tensor("x", [NT, 128, 2, D], BF16, kind="ExternalInput")
    mtw_d = nc.dram_tensor("mtw", [128, JJ, D], FP8, kind="ExternalInput")
    cv_d = nc.dram_tensor("cv", [1, D], BF16, kind="ExternalInput")
    s1_d = nc.dram_tensor("s1", [128, 1], BF16, kind="ExternalInput")
    cb_d = nc.dram_tensor("cb", [NCORES + 1, 128], BF16, kind="ExternalInput")
    out_d = nc.dram_tensor("out", [NT, 128, 2, D], BF16, kind="ExternalOutput")

    wsync_in = nc.dram_tensor("wsync_in", [1, D], BF16)
    wsync_out = nc.dram_tensor("wsync_out", [NCORES, D], BF16, addr_space="Shared")
    y_bnc = nc.dram_tensor("y_bnc", [1, D], BF16)
    gath = nc.dram_tensor("gath", [NCORES, D], BF16, addr_space="Shared")

    AF = mybir.ActivationFunctionType
    ALU = mybir.AluOpType

    with tile.TileContext(nc) as tc:
        with (
            tc.tile_pool(name="xpool", bufs=NT) as xpool,
            tc.tile_pool(name="wpool", bufs=1) as wpool,
            tc.tile_pool(name="small", bufs=1) as small,
            tc.tile_pool(name="psA", bufs=1, space="PSUM") as psA,
            tc.tile_pool(name="psB", bufs=1, space="PSUM") as psB,
        ):
            # --- warm-up collective: ncfw wake + rank rendezvous, no deps ---
            nc.gpsimd.collective_compute(
                "AllGather",
                ALU.bypass,
                replica_groups=RG,
                ins=[wsync_in.ap().opt()],
                outs=[wsync_out.ap().opt()],
            )

            # --- load x first; everything else is off the critical path ---
            xs = []
            for j in range(NT):
                xt = xpool.tile([128, 2, D], BF16, tag="xt")
                nc.sync.dma_start(xt[:], x_d[j])
                xs.append(xt)

            # --- weight / small-input loads ---
            mtw = wpool.tile([128, JJ, D], FP8, tag="mtw")
            nc.sync.dma_start(mtw[:], mtw_d[:])
            # gather tile: rows 0..7 = gathered partial logits, row 8 = cvec
            g2 = small.tile([NCORES + 1, D], BF16, tag="g2")
            nc.sync.dma_start(g2[NCORES : NCORES + 1, :], cv_d[:])

            # --- runtime power-of-2 scale constants (host-computed) ---
            ones1 = small.tile([128, 1], BF16, tag="ones1")
            nc.sync.dma_start(ones1[:], s1_d[:])
            comb9 = small.tile([NCORES + 1, 128], BF16, tag="comb9")
            nc.sync.dma_start(comb9[:], cb_d[:])
            # pre-warm the ScalarE sigmoid LUT off the critical path
            dummy = small.tile([1, 1], FP32, tag="dummy")
            nc.scalar.activation(dummy[:], ones1[0:1, 0:1], AF.Sigmoid)

            # --- accumulate token-sums on VectorE (bf16) as tiles land ---
            acc = wpool.tile([128, 2, D], BF16, tag="acc")
            nc.vector.tensor_copy(acc[:], xs[0][:])
            for j in range(1, NT):
                nc.vector.tensor_add(acc[:], acc[:], xs[j][:])
            acc2 = wpool.tile([128, D], BF16, tag="acc2")
            nc.vector.tensor_add(acc2[:], acc[:, 0, :], acc[:, 1, :])

            # partition-reduce: sumT[p, j] = sum_p' acc2[p', j*128+p]
            sumT_ps = psB.tile([128, JJ], FP32, tag="pt")
            for j in range(JJ):
                nc.tensor.matmul(
                    sumT_ps[:, j : j + 1],
                    acc2[:, j * 128 : (j + 1) * 128],
                    ones1[:],
                    start=True,
                    stop=True,
                )
            sT8 = small.tile([128, JJ], FP8, tag="sT8")
            nc.vector.tensor_copy(sT8[:], sumT_ps[:])

            # --- full fp8 matvec: y_c = M' @ s_c  (M' = mscale * M) ---
            y_ps = psA.tile([1, D], FP32, tag="wide")
            for j in range(JJ):
                for q in range(4):
                    nc.tensor.matmul(
                        y_ps[0:1, q * 512 : (q + 1) * 512],
                        sT8[:, j : j + 1],
                        mtw[:, j, q * 512 : (q + 1) * 512],
                        start=(j == 0),
                        stop=(j == JJ - 1),
                    )
            ybf = small.tile([1, D], BF16, tag="ybf")
            nc.scalar.copy(ybf[:], y_ps[:])
            nc.sync.dma_start(y_bnc[:], ybf[:])

            # --- the single data collective: gather partial logits ---
            nc.gpsimd.collective_compute(
                "AllGather",
                ALU.bypass,
                replica_groups=RG,
                ins=[y_bnc.ap().opt()],
                outs=[gath.ap().opt()],
            )
            nc.sync.dma_start(g2[0:4, :], gath[0:4, :])
            nc.sync.dma_start(g2[4:NCORES, :], gath[4:NCORES, :])

            # --- fused descale + rank-sum + cvec add + partition-broadcast:
            #     logit[p, n] = descale * sum_r g2[r, n] + cvec[n] ---
            logit_ps = psA.tile([128, D], FP32, tag="wide")
            for q in range(4):
                nc.tensor.matmul(
                    logit_ps[:, q * 512 : (q + 1) * 512],
                    comb9[:],
                    g2[:, q * 512 : (q + 1) * 512],
                    start=True,
                    stop=True,
                )
            gain_bc = wpool.tile([128, D], BF16, tag="gbc")
            for q in range(4):
                nc.scalar.activation(
                    gain_bc[:, q * 512 : (q + 1) * 512],
                    logit_ps[:, q * 512 : (q + 1) * 512],
                    AF.Sigmoid,
                )

            # --- scale x in place (bf16) and stream out ---
            for j in range(NT):
                for c2 in range(2):
                    nc.vector.tensor_mul(
                        xs[j][:, c2, :], xs[j][:, c2, :], gain_bc[:]
                    )
                nc.sync.dma_start(out_d[j], xs[j][:])

    nc.compile()
    return nc


def _get_nc():
    if "nc" not in _NC_CACHE:
        _NC_CACHE["nc"] = _build()
    return _NC_CACHE["nc"]


def _shard_inputs(x, state, proj_w, proj_b, gate_w, gate_b):
    import ml_dtypes

    bf16 = ml_dtypes.bfloat16
    fp8 = ml_dtypes.float8_e4m3
    x = np.asarray(x, dtype=np.float32)
    state = np.asarray(state, dtype=np.float32)
    proj_w = np.asarray(proj_w, dtype=np.float32)
    proj_b = np.asarray(proj_b, dtype=np.float32)
    gate_w = np.asarray(gate_w, dtype=np.float32)
    gate_b = np.asarray(gate_b, dtype=np.float32)

    # fold the two matvecs + EMA into one matrix and one bias vector
    M = (gate_w @ proj_w) * ((1.0 - DECAY) / float(B * T))
    cvec = gate_w @ (DECAY * state + (1.0 - DECAY) * proj_b) + gate_b

    # power-of-2 scales keeping the fp8 operands inside e4m3 range (+-448)
    max_m = float(np.abs(M).max()) + 1e-300
    mscale = 2.0 ** np.floor(np.log2(300.0 / max_m))
    max_s = float(np.abs(x.sum(axis=1)).max()) + 1e-300
    s1val = 2.0 ** min(0.0, np.floor(np.log2(300.0 / (1.25 * max_s))))

    # mtw[p, j, n] = (mscale * M)[n, 128j + p]
    mtw = np.ascontiguousarray(
        (M * mscale).T.reshape(JJ, 128, D).transpose(1, 0, 2).astype(fp8)
    )
    cv = np.ascontiguousarray(cvec.reshape(1, D).astype(bf16))
    s1 = np.full((128, 1), s1val, dtype=bf16)
    cb = np.empty((NCORES + 1, 128), dtype=bf16)
    cb[0:NCORES, :] = bf16(1.0 / (mscale * s1val))
    cb[NCORES, :] = bf16(1.0)

    in_maps = []
    for c in range(NCORES):
        xc = np.ascontiguousarray(x[c].reshape(NT, 128, 2, D).astype(bf16))
        in_maps.append({"x": xc, "mtw": mtw, "cv": cv, "s1": s1, "cb": cb})
    return in_maps


def _run(inputs, trace=False, **kwargs):
    nc = _get_nc()
    in_maps = _shard_inputs(**inputs)
    res = run_bass_kernel_spmd(
        nc, in_maps, core_ids=list(range(NCORES)), trace=trace, **kwargs
    )
    out = np.stack(
        [
            res.results[c]["out"].reshape(T, D).astype(np.float32)
            for c in range(NCORES)
        ],
        axis=0,
    )
    return out, res


def kernel(**inputs):
    out, _ = _run(inputs, trace=False)
    return out
